# revision 20
# baseline (speedup 1.0000x reference)
"""HardMemory retrieval-KNN kernel for 8 Trainium2 NeuronCores.

Data-parallel: 32 batches sharded 4-per-core; memory bank [1024,512]
replicated. The output is sparse by construction (a pixel contributes
only when its best cosine sim exceeds 0.8), so the kernel splits:

  device: simT[m,n] = <x_n, 16*mem_m/||mem_m||> via fp8(e4m3) DoubleRow
          matmuls (2x PE rate). Per 512-pixel strip the 1024 memory
          rows land in four [128,1024] PSUM tiles (two m-tiles each).
          ScalarE drains three of them to bf16 SBUF (it sits closest
          to PSUM, and a DVE op may read at most ONE operand from
          PSUM); the DVE drains the fourth fused with a max, plus a
          short bf16 2x tree, yielding a [128, 2, n] partial max over
          memory rows. Host finishes the max over those 256 lanes.
  host:   pixels with mx > 0.75*16*||x_n|| are candidates
          (conservative vs the true 0.8 threshold; the fp8 error
          budget is ~1 in these units, the margin is ~18). Candidates
          are rescored exactly in f32 (argmax + 0.8 mask), so the
          kernel is exact for any input; for typical inputs no pixel
          passes and the output is all zeros.
"""

import os
import sys

# Defensive: recover from a wedged NeuronCore left by a prior process.
# Read at runtime init; costs only startup wall time on a healthy device.
os.environ.setdefault("NEURON_RT_RESET_CORES", "1")

for _p in ("/opt/trn_rl_repo",):
    if _p not in sys.path:
        sys.path.insert(0, _p)

from contextlib import ExitStack

import ml_dtypes
import numpy as np

import concourse.bass as bass
import concourse.tile as tile
from concourse import bacc, mybir
from concourse.bass_utils import run_bass_kernel_spmd

F32 = mybir.dt.float32
BF16 = mybir.dt.bfloat16
FP8 = mybir.dt.float8e4
AF = mybir.ActivationFunctionType
ALU = mybir.AluOpType
PM = mybir.MatmulPerfMode

B_FULL, C, H, W = 32, 512, 64, 64
N_PIX = H * W
M = 1024
N_CORES = 8
B_LOC = B_FULL // N_CORES
KC = C // 256            # 2 DoubleRow contraction chunks (256 deep each)
MC = M // 128            # 8 memory chunks of 128 rows
MEM_SCALE = 16.0         # fp8 range centering for the normalized bank
DEV_THRESH = 0.75        # conservative candidate threshold (true: 0.8)
THRESHOLD = 0.8
EPS = 1e-12


def build_kernel(b_loc=B_LOC, n_pix=N_PIX):
    ns_count = n_pix // 512

    nc = bacc.Bacc("TRN2", target_bir_lowering=False, debug=False,
                   num_devices=N_CORES)
    xq = nc.dram_tensor("xq", [b_loc, KC, 128, 2, n_pix], FP8,
                        kind="ExternalInput")
    memq = nc.dram_tensor("memq", [128, KC * 2, M], FP8,
                          kind="ExternalInput")
    out_cm = nc.dram_tensor("out_cm", [b_loc, 128, 2, n_pix], BF16,
                            kind="ExternalOutput")

    with tile.TileContext(nc) as tc, ExitStack() as ctx:
        const = ctx.enter_context(tc.tile_pool(name="const", bufs=1))
        xio = ctx.enter_context(tc.tile_pool(name="xio", bufs=6))
        sdr = ctx.enter_context(tc.tile_pool(name="sdr", bufs=4))
        tdr = ctx.enter_context(tc.tile_pool(name="tdr", bufs=4))
        cmop = ctx.enter_context(tc.tile_pool(name="cmo", bufs=4))
        psim = ctx.enter_context(
            tc.tile_pool(name="psim", bufs=4, space=bass.MemorySpace.PSUM))

        memt = const.tile([128, KC * 2, M], FP8, tag="memt", name="memt")
        nc.sync.dma_start(memt[:, :, 0:M // 2], memq[:, :, 0:M // 2])
        nc.sync.dma_start(memt[:, :, M // 2:M], memq[:, :, M // 2:M])

        # PE warm-up: matmuls on memset-zero tiles need no DMA, so they
        # run during the preamble/DMA window and absorb the cold-clock
        # penalty that otherwise slows the first real matmuls.
        ws = const.tile([128, 2, 128], FP8, tag="ws", name="ws")
        nc.gpsimd.memset(ws[:], 0.0)
        wm = const.tile([128, 2, 512], FP8, tag="wm", name="wm")
        nc.gpsimd.memset(wm[:], 0.0)
        warm = psim.tile([128, 1024], F32, tag="big", name="big")
        for _ in range(6):
            nc.tensor.matmul(warm[:, 0:512], ws[:], wm[:],
                             start=True, stop=True,
                             perf_mode=PM.DoubleRow)

        xb = {}

        def load_x(b, slabs):
            for kc in range(KC):
                xb[b, kc] = xio.tile([128, 2, n_pix], FP8, tag=f"xq{kc}",
                                     name=f"xq{kc}")
            step = n_pix // slabs
            for s in range(slabs):
                for kc in range(KC):
                    nc.sync.dma_start(
                        xb[b, kc][:, :, s * step:(s + 1) * step],
                        xq[b, kc][:, :, s * step:(s + 1) * step])

        load_x(0, slabs=4)
        for b in range(b_loc):
            for ns in range(ns_count):
                if ns == min(2, ns_count - 1) and b + 1 < b_loc:
                    load_x(b + 1, slabs=1)
                nsl = slice(ns * 512, (ns + 1) * 512)
                big = [psim.tile([128, 1024], F32, tag="big", name="big")
                       for _ in range(4)]
                drains = {}
                for mt in range(MC):
                    dst = big[mt // 2][:, (mt % 2) * 512:(mt % 2) * 512 + 512]
                    for kc in range(KC):
                        nc.tensor.matmul(
                            dst,
                            memt[:, 2 * kc:2 * kc + 2,
                                 mt * 128:(mt + 1) * 128],
                            xb[b, kc][:, :, nsl],
                            start=(kc == 0), stop=(kc == KC - 1),
                            perf_mode=PM.DoubleRow)
                    if mt == 1:
                        s0 = sdr.tile([128, 1024], BF16, tag="s0", name="s0")
                        nc.scalar.activation(s0[:], big[0][:], AF.Copy)
                        drains["s0"] = s0
                    elif mt == 3:
                        s1 = sdr.tile([128, 1024], BF16, tag="s1", name="s1")
                        nc.scalar.activation(s1[:], big[1][:], AF.Copy)
                        u = tdr.tile([128, 1024], BF16, tag="u", name="u")
                        nc.vector.tensor_tensor(u[:], drains["s0"][:],
                                                s1[:], ALU.max)
                        drains["u"] = u
                    elif mt == 5:
                        s2 = sdr.tile([128, 1024], BF16, tag="s2", name="s2")
                        nc.scalar.activation(s2[:], big[2][:], AF.Copy)
                        drains["s2"] = s2
                    elif mt == 7:
                        uu = tdr.tile([128, 1024], BF16, tag="uu", name="uu")
                        nc.vector.tensor_tensor(uu[:], big[3][:],
                                                drains["s2"][:], ALU.max)
                        v = cmop.tile([128, 1024], BF16, tag="cmo",
                                      name="cmo")
                        nc.vector.tensor_tensor(v[:], drains["u"][:], uu[:],
                                                ALU.max)
                        nc.sync.dma_start(out_cm[b, :, :, nsl], v[:])

    nc.compile()
    return nc


_NC_CACHE = {}


def _get_nc(b_loc=B_LOC, n_pix=N_PIX):
    key = (b_loc, n_pix)
    if key not in _NC_CACHE:
        _NC_CACHE[key] = build_kernel(*key)
    return _NC_CACHE[key]


def host_prep(x_flat, memory):
    """x_flat [B, C, N] f32 -> (xq fp8 [B, KC, 128, 2, N], memq fp8
    [KC, 128, 2, M], mem_n [M, C] f32 row-normalized)."""
    B, C_, N = x_flat.shape
    mem_norms = np.maximum(np.linalg.norm(memory, axis=1), EPS)
    mem_n = memory / mem_norms[:, None]
    memT = mem_n.T * MEM_SCALE                       # [C, M]
    memq = np.ascontiguousarray(
        memT.reshape(KC, 2, 128, M).transpose(2, 0, 1, 3).reshape(
            128, KC * 2, M))
    memq = np.clip(memq, -240.0, 240.0).astype(ml_dtypes.float8_e4m3)
    xq = np.ascontiguousarray(
        x_flat.reshape(B, KC, 2, 128, N).transpose(0, 1, 3, 2, 4))
    xq = np.clip(xq, -240.0, 240.0).astype(ml_dtypes.float8_e4m3)
    return xq, memq, mem_n


def host_post(x_flat, memory, mem_n, cm_all):
    """cm_all [B, 128, 2, N] bf16 partial col-max -> out [B, C, N] f32."""
    B, C_, N = x_flat.shape
    mx = cm_all.astype(np.float32).max(axis=(1, 2))  # [B, N] scaled raw max
    xsq = np.einsum('bcn,bcn->bn', x_flat, x_flat, optimize=True)
    xnorm = np.maximum(np.sqrt(xsq), EPS)            # [B, N]
    cand = mx > (DEV_THRESH * MEM_SCALE) * xnorm
    out = np.zeros((B, C_, N), np.float32)
    if cand.any():
        bb, nn_ = np.nonzero(cand)
        xc = x_flat[bb, :, nn_]                      # [K, C]
        xcn = xc / np.maximum(
            np.linalg.norm(xc, axis=1, keepdims=True), EPS)
        sims = xcn @ mem_n.T                         # [K, M] exact f32
        mi = sims.argmax(axis=1)
        mv = sims.max(axis=1)
        sel = memory[mi] * (mv > THRESHOLD)[:, None]
        out[bb, :, nn_] = sel
    return out


def run_on_hw(x_flat, memory, b_loc=B_LOC, n_pix=N_PIX, trace=False,
              **spmd_kwargs):
    """x_flat: [N_CORES*b_loc, C, n_pix] f32. Returns (out_full, results)."""
    nc = _get_nc(b_loc, n_pix)
    xq, memq, mem_n = host_prep(x_flat, memory)
    in_maps = [
        {
            "xq": np.ascontiguousarray(xq[c * b_loc:(c + 1) * b_loc]),
            "memq": memq,
        }
        for c in range(N_CORES)
    ]
    res = run_bass_kernel_spmd(nc, in_maps, list(range(N_CORES)),
                               trace=trace, **spmd_kwargs)
    cm_all = np.concatenate(
        [res.results[c]["out_cm"] for c in range(N_CORES)], axis=0)
    out = host_post(x_flat, memory, mem_n, cm_all)
    return out, res


def kernel(x, memory):
    x = np.asarray(x, dtype=np.float32)
    memory = np.asarray(memory, dtype=np.float32)
    B, C_, H_, W_ = x.shape
    x_flat = np.ascontiguousarray(x.reshape(B, C_, H_ * W_))
    out_flat, _ = run_on_hw(x_flat, memory)
    return out_flat.reshape(B, C_, H_, W_)


# revision 27
# speedup vs baseline: 1.4252x; 1.4252x over previous
"""HardMemory retrieval-KNN kernel for 8 Trainium2 NeuronCores.

Data-parallel: 32 batches sharded 4-per-core; memory bank [1024,512]
replicated. The output is sparse by construction (a pixel contributes
only when its best cosine sim exceeds 0.8), so the kernel splits:

  device: simT[m,n] = <x_n, 16*mem_m/||mem_m||> via fp8(e4m3) DoubleRow
          matmuls (2x PE rate). Per 512-pixel strip the 1024 memory
          rows land in four [128,1024] PSUM tiles (two m-tiles each).
          ScalarE drains three of them to bf16 SBUF (it sits closest
          to PSUM, and a DVE op may read at most ONE operand from
          PSUM); the DVE drains the fourth fused with a max, plus a
          short bf16 2x tree, yielding a [128, 2, n] partial max over
          memory rows. Host finishes the max over those 256 lanes.
  host:   pixels with mx > 0.75*16*||x_n|| are candidates
          (conservative vs the true 0.8 threshold; the fp8 error
          budget is ~1 in these units, the margin is ~18). Candidates
          are rescored exactly in f32 (argmax + 0.8 mask), so the
          kernel is exact for any input; for typical inputs no pixel
          passes and the output is all zeros.
"""

import os
import sys

# Defensive: recover from a wedged NeuronCore left by a prior process.
# Read at runtime init; costs only startup wall time on a healthy device.
os.environ.setdefault("NEURON_RT_RESET_CORES", "1")

for _p in ("/opt/trn_rl_repo",):
    if _p not in sys.path:
        sys.path.insert(0, _p)

from contextlib import ExitStack

import ml_dtypes
import numpy as np

import concourse.bass as bass
import concourse.tile as tile
from concourse import bacc, mybir
from concourse.bass_utils import run_bass_kernel_spmd

F32 = mybir.dt.float32
BF16 = mybir.dt.bfloat16
FP8 = mybir.dt.float8e4
AF = mybir.ActivationFunctionType
ALU = mybir.AluOpType
PM = mybir.MatmulPerfMode

B_FULL, C, H, W = 32, 512, 64, 64
N_PIX = H * W
M = 1024
N_CORES = 8
B_LOC = B_FULL // N_CORES
C_DEV = 256              # channels screened on device (single DoubleRow k)
MC = M // 128            # 8 memory chunks of 128 rows
MEM_SCALE = 16.0         # fp8 range centering for the normalized bank
DEV_THRESH = 0.75        # conservative candidate threshold (true: 0.8)
SLACK = 6.0              # fp8-noise allowance on the screening bound
THRESHOLD = 0.8
EPS = 1e-12


def build_kernel(b_loc=B_LOC, n_pix=N_PIX):
    ns_count = n_pix // 512

    nc = bacc.Bacc("TRN2", target_bir_lowering=False, debug=False,
                   num_devices=N_CORES)
    xq = nc.dram_tensor("xq", [b_loc, 128, 2, n_pix], FP8,
                        kind="ExternalInput")
    memq = nc.dram_tensor("memq", [128, 2, M], FP8,
                          kind="ExternalInput")
    out_cm = nc.dram_tensor("out_cm", [b_loc, 128, 4, n_pix], BF16,
                            kind="ExternalOutput")

    with tile.TileContext(nc) as tc, ExitStack() as ctx:
        const = ctx.enter_context(tc.tile_pool(name="const", bufs=1))
        xio = ctx.enter_context(tc.tile_pool(name="xio", bufs=6))
        sdr = ctx.enter_context(tc.tile_pool(name="sdr", bufs=4))
        tdr = ctx.enter_context(tc.tile_pool(name="tdr", bufs=4))
        cmop = ctx.enter_context(tc.tile_pool(name="cmo", bufs=4))
        psim = ctx.enter_context(
            tc.tile_pool(name="psim", bufs=4, space=bass.MemorySpace.PSUM))

        memt = const.tile([128, 2, M], FP8, tag="memt", name="memt")
        nc.sync.dma_start(memt[:, :, 0:M // 2], memq[:, :, 0:M // 2])
        nc.sync.dma_start(memt[:, :, M // 2:M], memq[:, :, M // 2:M])

        # PE warm-up: matmuls on memset-zero tiles need no DMA, so they
        # run during the preamble/DMA window and absorb the cold-clock
        # penalty that otherwise slows the first real matmuls.
        ws = const.tile([128, 2, 128], FP8, tag="ws", name="ws")
        nc.gpsimd.memset(ws[:], 0.0)
        wm = const.tile([128, 2, 512], FP8, tag="wm", name="wm")
        nc.gpsimd.memset(wm[:], 0.0)
        warm = psim.tile([128, 1024], F32, tag="big", name="big")
        for _ in range(6):
            nc.tensor.matmul(warm[:, 0:512], ws[:], wm[:],
                             start=True, stop=True,
                             perf_mode=PM.DoubleRow)

        xb = {}

        def load_x(b, slabs):
            xb[b] = xio.tile([128, 2, n_pix], FP8, tag="xqt", name="xqt")
            step = n_pix // slabs
            for s in range(slabs):
                nc.sync.dma_start(
                    xb[b][:, :, s * step:(s + 1) * step],
                    xq[b][:, :, s * step:(s + 1) * step])

        load_x(0, slabs=4)
        for b in range(b_loc):
            for ns in range(ns_count):
                if ns == min(2, ns_count - 1) and b + 1 < b_loc:
                    load_x(b + 1, slabs=1)
                nsl = slice(ns * 512, (ns + 1) * 512)
                big = [psim.tile([128, 1024], F32, tag="big", name="big")
                       for _ in range(4)]
                drains = {}
                for mt in range(MC):
                    dst = big[mt // 2][:, (mt % 2) * 512:(mt % 2) * 512 + 512]
                    nc.tensor.matmul(
                        dst,
                        memt[:, :, mt * 128:(mt + 1) * 128],
                        xb[b][:, :, nsl],
                        start=True, stop=True,
                        perf_mode=PM.DoubleRow)
                    if mt == 1:
                        s0 = sdr.tile([128, 1024], BF16, tag="s0", name="s0")
                        nc.scalar.activation(s0[:], big[0][:], AF.Copy)
                        drains["s0"] = s0
                    elif mt == 3:
                        s1 = sdr.tile([128, 1024], BF16, tag="s1", name="s1")
                        nc.scalar.activation(s1[:], big[1][:], AF.Copy)
                        drains["s1"] = s1
                    elif mt == 5:
                        w = tdr.tile([128, 1024], BF16, tag="w", name="w")
                        nc.vector.tensor_tensor(w[:], big[2][:],
                                                drains["s0"][:], ALU.max)
                        nc.sync.dma_start(out_cm[b, :, 0:2, nsl], w[:])
                    elif mt == 7:
                        v = cmop.tile([128, 1024], BF16, tag="cmo",
                                      name="cmo")
                        nc.vector.tensor_tensor(v[:], big[3][:],
                                                drains["s1"][:], ALU.max)
                        nc.sync.dma_start(out_cm[b, :, 2:4, nsl], v[:])

    nc.compile()
    return nc


_NC_CACHE = {}


def _get_nc(b_loc=B_LOC, n_pix=N_PIX):
    key = (b_loc, n_pix)
    if key not in _NC_CACHE:
        _NC_CACHE[key] = build_kernel(*key)
    return _NC_CACHE[key]


def host_prep(x_flat, memory):
    """x_flat [B, C, N] f32 -> (xq fp8 [B, KC, 128, 2, N], memq fp8
    [KC, 128, 2, M], mem_n [M, C] f32 row-normalized)."""
    B, C_, N = x_flat.shape
    mem_norms = np.maximum(np.linalg.norm(memory, axis=1), EPS)
    mem_n = memory / mem_norms[:, None]
    memT = mem_n.T[:C_DEV] * MEM_SCALE               # [C_DEV, M]
    memq = np.ascontiguousarray(
        memT.reshape(2, 128, M).transpose(1, 0, 2))
    memq = np.clip(memq, -240.0, 240.0).astype(ml_dtypes.float8_e4m3)
    xq = np.ascontiguousarray(
        x_flat[:, :C_DEV].reshape(B, 2, 128, N).transpose(0, 2, 1, 3))
    xq = np.clip(xq, -240.0, 240.0).astype(ml_dtypes.float8_e4m3)
    return xq, memq, mem_n


def host_post(x_flat, memory, mem_n, cm_all):
    """cm_all [B, 128, 4, N] bf16 partial col-max over the first C_DEV
    channels -> out [B, C, N] f32. A pixel is a candidate when the
    rigorous tail bound  s1_max + SCALE*T*||x_tail|| + SLACK  could
    reach the (conservative) threshold; candidates are rescored
    exactly over all C channels."""
    B, C_, N = x_flat.shape
    mx = cm_all.astype(np.float32).max(axis=(1, 2))  # [B, N] scaled s1 max
    xsq = np.einsum('bcn,bcn->bn', x_flat, x_flat, optimize=True)
    xnorm = np.maximum(np.sqrt(xsq), EPS)            # [B, N]
    tail = x_flat[:, C_DEV:]
    xt = np.sqrt(np.einsum('bcn,bcn->bn', tail, tail, optimize=True))
    T = np.linalg.norm(mem_n[:, C_DEV:], axis=1).max()
    cand = mx > (DEV_THRESH * MEM_SCALE) * xnorm - MEM_SCALE * T * xt - SLACK
    out = np.zeros((B, C_, N), np.float32)
    if cand.any():
        bb, nn_ = np.nonzero(cand)
        xc = x_flat[bb, :, nn_]                      # [K, C]
        xcn = xc / np.maximum(
            np.linalg.norm(xc, axis=1, keepdims=True), EPS)
        sims = xcn @ mem_n.T                         # [K, M] exact f32
        mi = sims.argmax(axis=1)
        mv = sims.max(axis=1)
        sel = memory[mi] * (mv > THRESHOLD)[:, None]
        out[bb, :, nn_] = sel
    return out


def run_on_hw(x_flat, memory, b_loc=B_LOC, n_pix=N_PIX, trace=False,
              **spmd_kwargs):
    """x_flat: [N_CORES*b_loc, C, n_pix] f32. Returns (out_full, results)."""
    nc = _get_nc(b_loc, n_pix)
    xq, memq, mem_n = host_prep(x_flat, memory)
    in_maps = [
        {
            "xq": np.ascontiguousarray(xq[c * b_loc:(c + 1) * b_loc]),
            "memq": memq,
        }
        for c in range(N_CORES)
    ]
    res = run_bass_kernel_spmd(nc, in_maps, list(range(N_CORES)),
                               trace=trace, **spmd_kwargs)
    cm_all = np.concatenate(
        [res.results[c]["out_cm"] for c in range(N_CORES)], axis=0)
    out = host_post(x_flat, memory, mem_n, cm_all)
    return out, res


def kernel(x, memory):
    x = np.asarray(x, dtype=np.float32)
    memory = np.asarray(memory, dtype=np.float32)
    B, C_, H_, W_ = x.shape
    x_flat = np.ascontiguousarray(x.reshape(B, C_, H_ * W_))
    out_flat, _ = run_on_hw(x_flat, memory)
    return out_flat.reshape(B, C_, H_, W_)
